# revision 1
# baseline (speedup 1.0000x reference)
"""Bidirectional cross-attention + conv fusion block on 8 Trainium2 NeuronCores.

Sharding: data-parallel over the 8 independent (sample, direction) attention
units — core c handles sample c//2, direction c%2 (0 = s2-query, 1 = dem-query).
After attention + channel-LayerNorm, core pairs AllGather their LN outputs
(= the channel concat) in four j-chunks so the 3x3 conv can start while later
chunks are still in flight; BatchNorm statistics are AllReduced across one
core per sample, and each core finishes BN + ReLU + 1x1 conv for its sample.
Host takes even cores' outputs.

Key algebraic folds (all host-precomputed):
 - Q-projection is folded into the logits matmul: logits = K''^T xa_aug where
   K''[0:C] = (Wk^T Wq)-projected xb + Wq^T bk, K''[64] carries the
   per-key scalar (Wk^T bq).xb + bk.bq, and xa_aug has a trailing ones row.
   No Q tensor is ever materialized.
 - V bias rides an extra ones-contraction row (wva[64] = bv).
 - Softmax normalization is folded into V (v_i / Z_i), with Z from a 4x-mode
   DVE pass over the exp'd bf16 attention matrix (accum_out), not from the
   Act accumulator (saves 187ns x 128 on the bottleneck Act engine).
 - conv bias fb1 cancels exactly in train-mode BatchNorm and is dropped.
 - LN mean-subtraction is folded into a (I - 11^T/64) matmul; the residual
   add rides the otherwise-idle Pool engine.

The attention loop is software-pipelined with the exp stream lagging the
logits matmuls by one 1024-chunk, so the two rotating PSUM slots never stall
the Act engine; V projections are batched four i-blocks per PSUM grant.
PSUM banks are re-partitioned between loop and tail phases by closing the
loop pools (LN statistics take the logits banks, conv/final take the AV
accumulator banks).

Precision: fp32r for logits/LN/final matmuls; bf16 for the exp'd attention
matrix P, AV, and the 3x3 conv.  Softmax needs no max-subtraction: |logits|
<~ 1 by construction (weights ~N(0, 0.05^2)).
"""
import numpy as np
import ml_dtypes
from contextlib import ExitStack

import concourse.bass as bass
import concourse.tile as tile
from concourse import bacc, mybir
from concourse.bass_utils import run_bass_kernel_spmd

F32 = mybir.dt.float32
F32R = mybir.dt.float32r
BF16 = mybir.dt.bfloat16
Exp = mybir.ActivationFunctionType.Exp
Sqrt = mybir.ActivationFunctionType.Sqrt
Square = mybir.ActivationFunctionType.Square
Relu = mybir.ActivationFunctionType.Relu
MULT = mybir.AluOpType.mult
ADD = mybir.AluOpType.add
AX = mybir.AxisListType.X

B, C, H, W = 4, 64, 64, 64
HW = H * W            # 4096
N_CORES = 8
EPS_LN = 1e-5
EPS_BN = 1e-5
NI = HW // 128        # 32 i-blocks of 128
NJ = 4                # j-chunks of 1024 (AG granularity)
NT = HW // 512        # 8 j-tiles of 512
BN_COUNT = float(B * HW)

AG_GROUPS = [[0, 1], [2, 3], [4, 5], [6, 7]]
AR_GROUPS = [[0, 1, 2, 3, 4, 5, 6, 7]]
# overlapping conv-window row bands: conv group g depends on band g//2
WSTART = [0, 16]
WLEN = [18, 19]
NBAND = 2

_CACHE = {}


def _build(reps=1, fake_cc=False):
    nc = bacc.Bacc("TRN2", target_bir_lowering=False, debug=False,
                   num_devices=N_CORES)

    def din(name, shape, dt):
        return nc.dram_tensor(name, shape, dt, kind="ExternalInput").ap()

    xaq_d = din("xaq", [C, HW], F32R)        # query-side input (own direction)
    xkv_d = din("xkv", [C, HW], F32R)        # key/value-side input
    skp_d = din("skp", [65, 65], F32R)       # K'' projection stationary
    wva_d = din("wva", [65, C], F32R)        # V moving (wv.T rows + bv row)
    msub_d = din("msub", [C, C], F32R)       # I - 1/C  (mean-subtract matmul)
    lnm_d = din("lnm16", [C, C], BF16)       # all-1/C   (var-mean matmul)
    lng_d = din("lng", [C, 1], F32)
    lnb_d = din("lnb", [C, 1], F32)
    fw1_d = din("fw1t", [2 * C, 9 * C], BF16)  # conv w: [ic, tap*oc]
    bng_d = din("bng", [C, 1], F32)
    bnb_d = din("bnb", [C, 1], F32)
    fw2_d = din("fw2T", [C, C], F32R)        # fw2.T
    fb2_d = din("fb2", [C, 1], F32)

    out_d = nc.dram_tensor("out", [C, HW // 2], F32,
                           kind="ExternalOutput").ap()

    a2a_in = [nc.dram_tensor(f"agw_in{p}", [C, 2 * WLEN[p] * W], BF16).ap()
              for p in range(NBAND)]
    a2a_out = [nc.dram_tensor(f"agw_out{p}", [2 * C, 2 * WLEN[p] * W],
                              BF16).ap()
               for p in range(NBAND)]
    mka_d = nc.dram_tensor("mka", [2 * C, 1], mybir.dt.float32,
                           kind="ExternalInput").ap()
    mkb_d = nc.dram_tensor("mkb", [2 * C, 1], mybir.dt.float32,
                           kind="ExternalInput").ap()
    ar_in = nc.dram_tensor("ar_in", [C, 2], F32).ap()
    ar_out = nc.dram_tensor("ar_out", [C, 2], F32).ap()

    with tile.TileContext(nc) as tc:
        with ExitStack() as ctx:
            const = ctx.enter_context(tc.tile_pool(name="const", bufs=1))
            big = ctx.enter_context(tc.tile_pool(name="big", bufs=1))
            small = ctx.enter_context(tc.tile_pool(name="small", bufs=2))
            lps = ctx.enter_context(tc.tile_pool(name="lps", bufs=2, space="PSUM"))
            acc = ctx.enter_context(tc.tile_pool(name="acc", bufs=1, space="PSUM"))

            # ---- warm the Exp activation table while DMAs stream in ----
            warm = const.tile([1, 1], F32, tag="warm")
            nc.vector.memset(warm[:], 0.0)
            nc.scalar.activation(warm[:], warm[:], Exp)

            # ---- load inputs (K''-chain inputs first, tail weights last) ----
            skp = const.tile([65, 65], F32R, tag="skp")
            wva = const.tile([65, C], F32R, tag="wva")
            xa = const.tile([65, HW], F32R, tag="xa")    # query side + ones row
            xb = const.tile([65, HW], F32R, tag="xb")    # kv side + ones row
            nc.scalar.dma_start(skp[:], skp_d[:])
            nc.scalar.dma_start(wva[:], wva_d[:])
            nc.sync.dma_start(xb[0:C, 0:512], xkv_d[:, 0:512])
            nc.sync.dma_start(xb[0:C, 512:1024], xkv_d[:, 512:1024])
            nc.scalar.dma_start(xa[0:C, 0:512], xaq_d[:, 0:512])
            nc.scalar.dma_start(xa[0:C, 512:1024], xaq_d[:, 512:1024])
            nc.sync.dma_start(xa[0:C, 1024:2048], xaq_d[:, 1024:2048])
            nc.sync.dma_start(xa[0:C, 3072:4096], xaq_d[:, 3072:4096])
            nc.scalar.dma_start(xa[0:C, 2048:3072], xaq_d[:, 2048:3072])
            nc.scalar.dma_start(xb[0:C, 1024:2048], xkv_d[:, 1024:2048])
            nc.sync.dma_start(xb[0:C, 2048:3072], xkv_d[:, 2048:3072])
            nc.sync.dma_start(xb[0:C, 3072:4096], xkv_d[:, 3072:4096])
            # ones rows (memset as plain f32 -- the ISA has no f32r
            # set-value type): first xb tiles unblock the K'' chain
            xbo = xb[C:65, :].bitcast(F32)
            xao = xa[C:65, :].bitcast(F32)
            nc.gpsimd.memset(xbo[:, 0:512], 1.0)
            nc.gpsimd.memset(xbo[:, 512:1024], 1.0)
            nc.gpsimd.memset(xao[:, 0:2048], 1.0)
            nc.gpsimd.memset(xao[:, 2048:4096], 1.0)
            for qq in range(2, 8):
                nc.gpsimd.memset(xbo[:, qq * 512:(qq + 1) * 512], 1.0)

            lng = const.tile([C, 1], F32, tag="lng")
            lnb = const.tile([C, 1], F32, tag="lnb")
            nc.sync.dma_start(lng[:], lng_d[:])
            nc.sync.dma_start(lnb[:], lnb_d[:])
            fw1 = const.tile([2 * C, 9, C], BF16, tag="fw1")
            nc.sync.dma_start(fw1[:], fw1_d[:].rearrange("p (t o) -> p t o", t=9))
            msub = const.tile([C, C], F32R, tag="msub")
            lnm = const.tile([C, C], BF16, tag="lnm")
            nc.sync.dma_start(msub[:], msub_d[:])
            nc.sync.dma_start(lnm[:], lnm_d[:])
            bng = const.tile([C, 1], F32, tag="bng")
            bnb = const.tile([C, 1], F32, tag="bnb")
            fw2 = const.tile([C, C], F32R, tag="fw2")
            fb2 = const.tile([C, 1], F32, tag="fb2")
            nc.sync.dma_start(bng[:], bng_d[:])
            nc.sync.dma_start(bnb[:], bnb_d[:])
            nc.sync.dma_start(fw2[:], fw2_d[:])
            nc.sync.dma_start(fb2[:], fb2_d[:])

            eps = const.tile([C, 1], F32, tag="eps")
            nc.vector.memset(eps[:], EPS_LN)
            mka = const.tile([2 * C, 1], F32, tag="mka")
            mkb = const.tile([2 * C, 1], F32, tag="mkb")
            nc.sync.dma_start(mka[:], mka_d[:])
            nc.sync.dma_start(mkb[:], mkb_d[:])
            # zero padding rows of the conv windows: block0 w=0 (global -1)
            # and block1 w=33,34 (global 64,65)
            zr = const.tile([C, 128], BF16, tag="zr")
            nc.vector.memset(zr[:], 0.0)
            nc.sync.dma_start(a2a_in[0][:, 0:64], zr[:, 0:64])
            # band 1 block1: w=33,34 (global rows 64,65) at offsets 17,18
            nc.sync.dma_start(a2a_in[1][:, (19 + 17) * 64:(19 + 19) * 64],
                              zr[:])

            for rep in range(reps):
              actx = ExitStack()
              abig = actx.enter_context(tc.tile_pool(name=f"abig{rep}", bufs=1))
              ppool = actx.enter_context(tc.tile_pool(name=f"ppool{rep}", bufs=2))

              kpp = abig.tile([65, HW], F32R, tag="kpp")   # K'' [65, i]

              def emit_kproj(jt, n=1):   # K'' j-tiles jt..jt+n-1 (one grant)
                  sl = slice(jt * 512, (jt + n) * 512)
                  pk = lps.tile([65, 512 * n], F32, tag="lgt", name=f"pk{jt}")
                  for q in range(n):
                      nc.tensor.matmul(
                          pk[:, q * 512:(q + 1) * 512], skp[:],
                          xb[:, (jt + q) * 512:(jt + q + 1) * 512])
                  nc.vector.tensor_scalar(kpp[:, sl], pk[:], 1.0, 0.0, MULT, ADD)

              vtab = {}   # i-block -> (tile, col0)

              def emit_pv(b0, n):   # V for i-blocks b0..b0+n-1 (one grant)
                  pvp = lps.tile([128, 64 * n], F32, tag="lgt", name=f"pv{b0}")
                  for q in range(n):
                      ibb = b0 + q
                      nc.tensor.matmul(pvp[:, q * 64:(q + 1) * 64],
                                       xb[:, ibb * 128:(ibb + 1) * 128], wva[:])
                  vt = small.tile([128, 64 * n], F32, tag=f"vt{n}")
                  nc.vector.tensor_scalar(vt[:], pvp[:], 1.0, 0.0, MULT, ADD)
                  for q in range(n):
                      vtab[b0 + q] = (vt, q * 64)

              accb = [acc.tile([128, 512], F32, tag=f"acc{jj}", name=f"acc{jj}")
                      for jj in range(4)]

              PAs = {}
              vss = {}
              hold = {}

              def emit_qk(k, ch):
                  isl = slice(k * 128, (k + 1) * 128)
                  ps = lps.tile([128, 1024], F32, tag="lgt", name=f"ps{k}_{ch}")
                  c0 = ch * 1024
                  for hh in range(2):
                      sl = slice(c0 + hh * 512, c0 + (hh + 1) * 512)
                      nc.tensor.matmul(ps[:, hh * 512:(hh + 1) * 512],
                                       kpp[:, isl], xa[:, sl])
                  return ps

              def emit_exp(k, ch, ps):
                  if ch == 0:
                      PAs[k] = ppool.tile([128, HW], BF16, tag="PA",
                                          name=f"PA{k}")
                  nc.scalar.activation(PAs[k][:, ch * 1024:(ch + 1) * 1024],
                                       ps[:], Exp, scale=0.125)
                  if ch == 3:
                      S = small.tile([128, 1], F32, tag="S")
                      nc.vector.tensor_scalar(PAs[k][:], PAs[k][:], 1.0, 0.0,
                                              MULT, ADD, accum_out=S[:])
                      hold["S"] = S
                      R = small.tile([128, 1], F32, tag="R")
                      nc.vector.reciprocal(R[:], S[:])
                      vs = small.tile([128, C], BF16, tag="vs")
                      vt, c0v = vtab[k]
                      nc.vector.tensor_scalar_mul(
                          vs[:], vt[:, c0v:c0v + 64], R[:])
                      vss[k] = vs

              def emit_av(k, ch, stop=False):
                  se = slice(ch * 1024, ch * 1024 + 512)
                  so = slice(ch * 1024 + 512, ch * 1024 + 1024)
                  first = (k == 0)
                  nc.tensor.matmul(accb[ch][0:C, :], vss[k][:], PAs[k][:, se],
                                   tile_position=(0, 0), start=first,
                                   stop=False)
                  nc.tensor.matmul(accb[ch][C:2 * C, :], vss[k][:],
                                   PAs[k][:, so],
                                   tile_position=(0, 64), start=first,
                                   stop=stop)

              emit_kproj(0)
              emit_kproj(1)
              emit_pv(0, 4)
              emit_pv(4, 4)

              pending = None
              for k in range(NI):
                  for ch in range(4):
                      ps = emit_qk(k, ch)
                      if pending is not None:
                          pk_, pch_ = pending[0], pending[1]
                          emit_exp(*pending)
                          if pk_ >= 1:
                              if pch_ == 0:
                                  emit_av(pk_ - 1, 0)
                              elif pch_ == 1:
                                  emit_av(pk_ - 1, 1)
                              elif pch_ == 2:
                                  emit_av(pk_ - 1, 2)
                                  emit_av(pk_ - 1, 3)
                      pending = (k, ch, ps)
                      if ch == 2 and k % 8 == 6 and k < 24:
                          t = k // 8
                          emit_kproj(2 * t + 2, n=2)
                          emit_pv(8 * t + 8, 8)
              emit_exp(*pending)
              # pre-swap the activation table to the sqrt set while the AV
              # drain runs, so LN's first Sqrt doesn't stall mid-pipeline.
              # Reading the last block's softmax sum pins this to the drain
              # (a dependency-free op would be scheduler-hoisted mid-loop).
              warm2 = small.tile([128, 1], F32, tag="warm2")
              nc.scalar.activation(warm2[:], hold["S"][:], Sqrt)
              for ch in range(4):
                  emit_av(NI - 1, ch, stop=True)

              actx.close()
              tctx = ExitStack()
              tmp = tctx.enter_context(tc.tile_pool(name=f"tmp{rep}", bufs=2))

              att = big.tile([C, HW], F32R, tag="att")

              # residual adds (psum + xa) must run on DVE (GPSIMD cannot
              # touch PSUM on real HW).  Emit 6,7 first so acc3 frees early
              # for the LN variance accumulator, then weave the rest between
              # LN chunks so the DVE FIFO stays fed.
              def emit_resid(jc):
                  jp, hh = jc // 2, jc % 2
                  s2 = slice(jc * 512, (jc + 1) * 512)
                  nc.vector.tensor_tensor(
                      att[:, s2], accb[jp][hh * C:(hh + 1) * C, :],
                      xa[0:C, s2], ADD)

              for jc in [6, 7, 0, 1]:
                  emit_resid(jc)

              cinb = [big.tile([2 * C, WLEN[p], W], BF16, tag=f"cin{p}",
                               name=f"cinb{p}")
                      for p in range(NBAND)]
              y = big.tile([C, HW // 2], BF16, tag="y")
              bnp4 = small.tile([C, 4], F32, tag="bnp4")
              bnq4 = small.tile([C, 4], F32, tag="bnq4")

              def emit_ln_half(jc):
                  sl = slice(jc * 512, (jc + 1) * 512)
                  pxm = acc.tile([C, 512], F32, tag=f"acc{jc % 3}",
                                 name=f"pxm{jc}")
                  nc.tensor.matmul(pxm[:], msub[:], att[:, sl])
                  sq2 = tmp.tile([C, 512], BF16, tag="sq2")
                  nc.scalar.activation(sq2[:], pxm[:], Square)
                  if jc % 3 == 0:
                      pe2 = acc.tile([C, 512], F32, tag="acc3",
                                     name=f"pe2{jc}")
                  else:
                      pe2 = lps.tile([C, 512], F32, tag="lgt",
                                     name=f"pe2{jc}")
                  nc.tensor.matmul(pe2[:], lnm[:], sq2[:])
                  sd = tmp.tile([C, 512], F32, tag="sd")
                  nc.scalar.activation(sd[:], pe2[:], Sqrt, bias=eps[:])
                  rstd = tmp.tile([C, 512], F32, tag="rstd")
                  nc.vector.reciprocal(rstd[:], sd[:])
                  xh = tmp.tile([C, 512], BF16, tag="xh")
                  nc.vector.tensor_tensor(xh[:], pxm[:], rstd[:], MULT)
                  oln = tmp.tile([C, 512], BF16, tag="oln")
                  nc.vector.tensor_scalar(oln[:], xh[:], lng[:], lnb[:],
                                          MULT, ADD)
                  # scatter this chunk of LN output into the AllToAll send
                  # buffer: block h = the 35-row conv window of rank h
                  # (window row w <-> global row 32h - 1 + w)
                  for h in range(2):
                      g0b = max(8 * jc, 0 if h == 0 else 31)
                      g1b = min(8 * jc + 8, 34 if h == 0 else 64)
                      for p in range(NBAND):
                          w0p, w1p = WSTART[p], WSTART[p] + WLEN[p]
                          gof = -1 if h == 0 else 31    # g = w + gof
                          g0 = max(g0b, w0p + gof)
                          g1 = min(g1b, w1p + gof)
                          if g0 >= g1:
                              continue
                          w0 = g0 - gof
                          base = h * WLEN[p] * 64 + (w0 - w0p) * 64
                          dst = a2a_in[p][:, base:base + (g1 - g0) * 64]
                          src = oln[:, (g0 - 8 * jc) * 64:
                                    (g1 - 8 * jc) * 64]
                          nc.sync.dma_start(dst, src)

              def emit_a2a(p):
                  if fake_cc:
                      nc.scalar.dma_start(a2a_out[p][0:C, :], a2a_in[p][:])
                      nc.sync.dma_start(a2a_out[p][C:2 * C, :], a2a_in[p][:])
                  else:
                      nc.gpsimd.collective_compute(
                          "AllGather", mybir.AluOpType.bypass,
                          replica_groups=AG_GROUPS,
                          ins=[a2a_in[p][:]], outs=[a2a_out[p][:]])

              wabs = {}

              def emit_wab_load(p):
                  wl64 = WLEN[p] * 64
                  wab = tmp.tile([2 * C, 2 * 19 * 64], BF16, tag="wab",
                                 name=f"wab{p}")
                  nc.scalar.dma_start(wab[:, 0:2 * wl64], a2a_out[p][:])
                  wabs[p] = wab

              def emit_cin_sel(p):
                  wl64 = WLEN[p] * 64
                  wab = wabs[p]
                  ta = tmp.tile([2 * C, 19 * 64], BF16, tag="ta")
                  nc.vector.tensor_scalar_mul(ta[:, 0:wl64],
                                              wab[:, 0:wl64], mka[:])
                  tb = tmp.tile([2 * C, 19 * 64], BF16, tag="tb")
                  nc.vector.tensor_scalar_mul(tb[:, 0:wl64],
                                              wab[:, wl64:2 * wl64], mkb[:])
                  cbf = cinb[p][:].rearrange("p h w -> p (h w)")
                  nc.vector.tensor_tensor(cbf[:, 0:wl64],
                                          ta[:, 0:wl64], tb[:, 0:wl64], ADD)

              TAPS = [(1, 1)] + [(ki, kj) for ki in range(3) for kj in range(3)
                                 if (ki, kj) != (1, 1)]

              def emit_conv_group(g):
                  pc = acc.tile([C, 8, W], F32, tag=f"acc{g}", name=f"pc{g}")
                  for t, (ki, kj) in enumerate(TAPS):
                      w_lo = max(0, 1 - kj)
                      w_hi = min(W, W + 1 - kj)
                      rhs = cinb[g // 2][:, 8 * (g % 2) + ki:
                                         8 * (g % 2) + ki + 8,
                                         w_lo + kj - 1:w_hi + kj - 1]
                      nc.tensor.matmul(pc[:, 0:8, w_lo:w_hi],
                                       fw1[:, 3 * ki + kj, :], rhs,
                                       start=(t == 0), stop=(t == 8))
                  return pc

              for jc in range(8):
                  emit_ln_half(jc)
                  if jc < 4:
                      emit_resid(jc + 2)
              for p in range(NBAND):
                  emit_a2a(p)
              for p in range(NBAND):
                  emit_wab_load(p)
              # PE p-state warm-up: dummy matmuls gated on the exchange
              # load, so the tensor engine is ramping exactly when the conv
              # input lands (otherwise conv starts at the slow pstate)
              for d in range(4):
                  pw = lps.tile([C, 512], F32, tag="lgt", name=f"pw{d}")
                  nc.tensor.matmul(pw[:], lnm[:],
                                   wabs[0][0:C, d * 512:(d + 1) * 512])
              for p in range(NBAND):
                  emit_cin_sel(p)
              for g in range(4):
                  pc = emit_conv_group(g)
                  ysl = y[:, g * 512:(g + 1) * 512]
                  nc.vector.tensor_scalar(
                      ysl, pc[:].rearrange("p r w -> p (r w)"), 1.0, 0.0,
                      MULT, ADD, accum_out=bnp4[:, g:g + 1])
                  ysq = tmp.tile([C, 512], BF16, tag="ysq")
                  nc.scalar.activation(ysq[:], ysl, Square,
                                       accum_out=bnq4[:, g:g + 1])

              # ---- BatchNorm stats (cross-sample AllReduce, all 8 cores
              # hold disjoint row-halves) ----
              bnp = small.tile([C, 2], F32, tag="bnp")
              nc.vector.tensor_reduce(bnp[:, 0:1], bnp4[:], AX, ADD)
              nc.vector.tensor_reduce(bnp[:, 1:2], bnq4[:], AX, ADD)
              nc.sync.dma_start(ar_in[:], bnp[:])
              if fake_cc:
                  nc.sync.dma_start(ar_out[:], ar_in[:])
              else:
                  nc.gpsimd.collective_compute(
                      "AllReduce", mybir.AluOpType.add,
                      replica_groups=AR_GROUPS,
                      ins=[ar_in[:]], outs=[ar_out[:]])
              bns = small.tile([C, 2], F32, tag="bns")
              nc.sync.dma_start(bns[:], ar_out[:])

              m2 = small.tile([C, 2], F32, tag="m2")
              nc.vector.tensor_scalar_mul(m2[:], bns[:], 1.0 / BN_COUNT)
              musq2 = small.tile([C, 1], F32, tag="musq2")
              nc.vector.tensor_mul(musq2[:], m2[:, 0:1], m2[:, 0:1])
              varb = small.tile([C, 1], F32, tag="varb")
              nc.vector.tensor_sub(varb[:], m2[:, 1:2], musq2[:])
              sdb = small.tile([C, 1], F32, tag="sdb")
              nc.scalar.activation(sdb[:], varb[:], Sqrt, bias=eps[:])
              rstdb = small.tile([C, 1], F32, tag="rstdb")
              nc.vector.reciprocal(rstdb[:], sdb[:])
              scl = small.tile([C, 1], F32, tag="scl")
              nc.vector.tensor_mul(scl[:], bng[:], rstdb[:])
              msc = small.tile([C, 1], F32, tag="msc")
              nc.vector.tensor_mul(msc[:], m2[:, 0:1], scl[:])
              shf = small.tile([C, 1], F32, tag="shf")
              nc.vector.tensor_sub(shf[:], bnb[:], msc[:])

              # ---- BN apply + ReLU + final 1x1 conv (own row-half) ----
              yr = big.tile([C, HW // 2], F32R, tag="yr")
              for hh in range(2):
                  sl = slice(hh * 1024, (hh + 1) * 1024)
                  nc.scalar.activation(yr[:, sl], y[:, sl], Relu,
                                       scale=scl[:], bias=shf[:])
                  po = lps.tile([C, 1024], F32, tag="lgt", name=f"po{hh}")
                  for q in range(2):
                      s2 = slice(hh * 1024 + q * 512,
                                 hh * 1024 + (q + 1) * 512)
                      nc.tensor.matmul(po[:, q * 512:(q + 1) * 512],
                                       fw2[:], yr[:, s2])
                  ot = tmp.tile([C, 1024], F32, tag="ot")
                  nc.vector.tensor_scalar_add(ot[:], po[:], fb2[:])
                  [nc.sync, nc.scalar][hh].dma_start(out_d[:, sl], ot[:])
              tctx.close()

    nc.compile()
    return nc


def _get_nc(reps=1, fake_cc=False):
    key = f"nc{reps}_{fake_cc}"
    if key not in _CACHE:
        _CACHE[key] = _build(reps=reps, fake_cc=fake_cc)
    return _CACHE[key]


def _make_in_maps(inputs):
    return _build_in_maps(**inputs)


def _build_in_maps(x_s2, x_dem, wq1, bq1, wk1, bk1, wv1, bv1,
                   wq2, bq2, wk2, bk2, wv2, bv2,
                   ln_s2_w, ln_s2_b, ln_dem_w, ln_dem_b,
                   fw1, fb1, bn_g, bn_b, fw2, fb2):
    f32 = np.float32
    x_s2 = np.asarray(x_s2, f32).reshape(B, C, HW)
    x_dem = np.asarray(x_dem, f32).reshape(B, C, HW)

    fw1t = np.ascontiguousarray(
        np.transpose(np.asarray(fw1, f32), (1, 2, 3, 0)).reshape(2 * C, 9 * C)
    ).astype(ml_dtypes.bfloat16)
    msub = (np.eye(C, dtype=f32) - np.full((C, C), 1.0 / C, f32))
    lnm16 = np.full((C, C), 1.0 / C, f32).astype(ml_dtypes.bfloat16)
    common = {
        "msub": msub,
        "lnm16": lnm16,
        "fw1t": fw1t,
        "bng": np.asarray(bn_g, f32).reshape(C, 1),
        "bnb": np.asarray(bn_b, f32).reshape(C, 1),
        "fw2T": np.ascontiguousarray(np.asarray(fw2, f32).T),
        "fb2": np.asarray(fb2, f32).reshape(C, 1),
    }

    def mk_dir(wq, bq, wk, bk, wv, bv, lg, lb):
        wq = np.asarray(wq, f32); wk = np.asarray(wk, f32)
        wv = np.asarray(wv, f32)
        bq = np.asarray(bq, f32).reshape(C)
        bk = np.asarray(bk, f32).reshape(C)
        bv = np.asarray(bv, f32).reshape(C)
        skp = np.zeros((65, 65), f32)
        skp[:C, :C] = wk.T @ wq
        skp[C, :C] = wq.T @ bk
        skp[:C, C] = wk.T @ bq
        skp[C, C] = bk @ bq
        wva = np.zeros((65, C), f32)
        wva[:C, :] = wv.T
        wva[C, :] = bv
        return dict(skp=skp, wva=wva,
                    lng=np.asarray(lg, f32).reshape(C, 1),
                    lnb=np.asarray(lb, f32).reshape(C, 1))

    dir_params = [
        mk_dir(wq1, bq1, wk1, bk1, wv1, bv1, ln_s2_w, ln_s2_b),
        mk_dir(wq2, bq2, wk2, bk2, wv2, bv2, ln_dem_w, ln_dem_b),
    ]
    in_maps = []
    for c in range(N_CORES):
        b, d = c // 2, c % 2
        xaq = x_s2[b] if d == 0 else x_dem[b]
        xkv = x_dem[b] if d == 0 else x_s2[b]
        m = {"xaq": np.ascontiguousarray(xaq),
             "xkv": np.ascontiguousarray(xkv),
             "mka": np.full((2 * C, 1), 1.0 - d, f32),
             "mkb": np.full((2 * C, 1), float(d), f32)}
        m.update(dir_params[d])
        m.update(common)
        in_maps.append(m)
    return in_maps


def kernel(**inputs):
    nc = _get_nc()
    in_maps = _make_in_maps(inputs)
    res = run_bass_kernel_spmd(nc, in_maps, list(range(N_CORES)))
    out = np.empty((B, C, H, W), np.float32)
    for b in range(B):
        half = np.concatenate([res.results[2 * b]["out"],
                               res.results[2 * b + 1]["out"]], axis=1)
        out[b] = half.reshape(C, H, W)
    return out



# revision 18
# speedup vs baseline: 1.0104x; 1.0104x over previous
"""Bidirectional cross-attention + conv fusion block on 8 Trainium2 NeuronCores.

Sharding: data-parallel over the 8 independent (sample, direction) attention
units — core c handles sample c//2, direction c%2 (0 = s2-query, 1 = dem-query).
After attention + channel-LayerNorm, core pairs AllGather their LN outputs
(= the channel concat) in four j-chunks so the 3x3 conv can start while later
chunks are still in flight; BatchNorm statistics are AllReduced across one
core per sample, and each core finishes BN + ReLU + 1x1 conv for its sample.
Host takes even cores' outputs.

Key algebraic folds (all host-precomputed):
 - Q-projection is folded into the logits matmul: logits = K''^T xa_aug where
   K''[0:C] = (Wk^T Wq)-projected xb + Wq^T bk, K''[64] carries the
   per-key scalar (Wk^T bq).xb + bk.bq, and xa_aug has a trailing ones row.
   No Q tensor is ever materialized.
 - V bias rides an extra ones-contraction row (wva[64] = bv).
 - Softmax normalization is folded into V (v_i / Z_i), with Z from a 4x-mode
   DVE pass over the exp'd bf16 attention matrix (accum_out), not from the
   Act accumulator (saves 187ns x 128 on the bottleneck Act engine).
 - conv bias fb1 cancels exactly in train-mode BatchNorm and is dropped.
 - LN mean-subtraction is folded into a (I - 11^T/64) matmul; the residual
   add rides the otherwise-idle Pool engine.

The attention loop is software-pipelined with the exp stream lagging the
logits matmuls by one 1024-chunk, so the two rotating PSUM slots never stall
the Act engine; V projections are batched four i-blocks per PSUM grant.
PSUM banks are re-partitioned between loop and tail phases by closing the
loop pools (LN statistics take the logits banks, conv/final take the AV
accumulator banks).

Precision: fp32r for logits/LN/final matmuls; bf16 for the exp'd attention
matrix P, AV, and the 3x3 conv.  Softmax needs no max-subtraction: |logits|
<~ 1 by construction (weights ~N(0, 0.05^2)).
"""
import numpy as np
import ml_dtypes
from contextlib import ExitStack

import concourse.bass as bass
import concourse.tile as tile
from concourse import bacc, mybir
from concourse.bass_utils import run_bass_kernel_spmd

F32 = mybir.dt.float32
F32R = mybir.dt.float32r
BF16 = mybir.dt.bfloat16
FP8 = mybir.dt.float8e4
DR = mybir.MatmulPerfMode.DoubleRow
Exp = mybir.ActivationFunctionType.Exp
Sqrt = mybir.ActivationFunctionType.Sqrt
Square = mybir.ActivationFunctionType.Square
Relu = mybir.ActivationFunctionType.Relu
MULT = mybir.AluOpType.mult
ADD = mybir.AluOpType.add
AX = mybir.AxisListType.X

B, C, H, W = 4, 64, 64, 64
HW = H * W            # 4096
N_CORES = 8
EPS_LN = 1e-5
EPS_BN = 1e-5
NI = HW // 128        # 32 i-blocks of 128
NJ = 4                # j-chunks of 1024 (AG granularity)
NT = HW // 512        # 8 j-tiles of 512
BN_COUNT = float(B * HW)

AG_GROUPS = [[0, 1], [2, 3], [4, 5], [6, 7]]
AR_GROUPS = [[0, 1, 2, 3, 4, 5, 6, 7]]
# overlapping conv-window row bands: conv group g depends on band g//2
WSTART = [0, 16]
WLEN = [18, 19]
NBAND = 2

_CACHE = {}


def _build(reps=1, fake_cc=False):
    nc = bacc.Bacc("TRN2", target_bir_lowering=False, debug=False,
                   num_devices=N_CORES)

    def din(name, shape, dt):
        return nc.dram_tensor(name, shape, dt, kind="ExternalInput").ap()

    # fp8 pair-layout operands for DoubleRow matmuls: logical contraction
    # channel c = s*33 + p lives at [partition p, slot s]; channels 0-63 are
    # the conv-map channels, 64 = ones (bias folds), 65 = zero padding.
    xa8_d = din("xa8", [33, 2 * HW], FP8)    # query-side input (own direction)
    xb8_d = din("xb8", [33, 2 * HW], FP8)    # key/value-side input
    xar_d = din("xar", [C, HW], F32)         # 4096*x_a for the residual add
    skp_d = din("skp8", [33, 2 * 96], FP8)   # 64*K'' projection stationary
    wva_d = din("wva8", [33, 2 * C], FP8)    # 16*wv.T (+bv row)
    msub_d = din("msub", [C, C], F32R)       # I - 1/C  (mean-subtract matmul)
    lnm_d = din("lnm16", [C, C], BF16)       # all-1/C   (var-mean matmul)
    lng_d = din("lng", [C, 1], F32)
    lnb_d = din("lnb", [C, 1], F32)
    fw1_d = din("fw1t", [2 * C, 9 * C], BF16)  # conv w: [ic, tap*oc]
    bng_d = din("bng", [C, 1], F32)
    bnb_d = din("bnb", [C, 1], F32)
    fw2_d = din("fw2T", [C, C], F32R)        # fw2.T
    fb2_d = din("fb2", [C, 1], F32)

    out_d = nc.dram_tensor("out", [C, HW // 2], F32,
                           kind="ExternalOutput").ap()

    a2a_in = [nc.dram_tensor(f"agw_in{p}", [C, 2 * WLEN[p] * W], BF16).ap()
              for p in range(NBAND)]
    a2a_out = [nc.dram_tensor(f"agw_out{p}", [2 * C, 2 * WLEN[p] * W],
                              BF16).ap()
               for p in range(NBAND)]
    mka_d = nc.dram_tensor("mka", [2 * C, 1], mybir.dt.float32,
                           kind="ExternalInput").ap()
    mkb_d = nc.dram_tensor("mkb", [2 * C, 1], mybir.dt.float32,
                           kind="ExternalInput").ap()
    ar_in = nc.dram_tensor("ar_in", [C, 2], F32).ap()
    ar_out = nc.dram_tensor("ar_out", [C, 2], F32).ap()

    with tile.TileContext(nc) as tc:
        with ExitStack() as ctx:
            const = ctx.enter_context(tc.tile_pool(name="const", bufs=1))
            big = ctx.enter_context(tc.tile_pool(name="big", bufs=1))
            small = ctx.enter_context(tc.tile_pool(name="small", bufs=2))
            lps = ctx.enter_context(tc.tile_pool(name="lps", bufs=2, space="PSUM"))
            acc = ctx.enter_context(tc.tile_pool(name="acc", bufs=1, space="PSUM"))

            # ---- warm the Exp activation table while DMAs stream in ----
            warm = const.tile([1, 1], F32, tag="warm")
            nc.vector.memset(warm[:], 0.0)
            nc.scalar.activation(warm[:], warm[:], Exp)

            # ---- load inputs (K''-chain inputs first, tail weights last) ----
            # K''-channel halves padded to 48 cols: fp8 dual-row LDWEIGHTS
            # requires the pair-dim byte stride and offsets to be 16-aligned
            skp = const.tile([33, 2, 96], FP8, tag="skp")
            wva = const.tile([33, 2, C], FP8, tag="wva")
            xa8 = const.tile([33, 2, HW], FP8, tag="xa8")
            xb8 = const.tile([33, 2, HW], FP8, tag="xb8")
            xar = const.tile([C, HW], F32, tag="xar")    # 4096*x_a residual
            nc.scalar.dma_start(skp[:], skp_d[:].rearrange("p (s n) -> p s n",
                                                           s=2))
            nc.scalar.dma_start(wva[:], wva_d[:].rearrange("p (s n) -> p s n",
                                                           s=2))
            xb8f = xb8[:].rearrange("p s n -> p (s n)")
            xa8f = xa8[:].rearrange("p s n -> p (s n)")
            # slot-major flat layout: slot s, col j at offset s*HW + j.
            # Load both slots of low j first (K'' j-tile 0, QK chunk 0).
            for s, lo, hi, q in [(0, 0, 1024, nc.sync), (1, 0, 1024, nc.sync),
                                 (0, 1024, HW, nc.scalar),
                                 (1, 1024, HW, nc.sync)]:
                q.dma_start(xb8f[:, s * HW + lo:s * HW + hi],
                            xb8_d[:, s * HW + lo:s * HW + hi])
            for s, lo, hi, q in [(0, 0, 1024, nc.scalar),
                                 (1, 0, 1024, nc.scalar),
                                 (0, 1024, HW, nc.sync),
                                 (1, 1024, HW, nc.scalar)]:
                q.dma_start(xa8f[:, s * HW + lo:s * HW + hi],
                            xa8_d[:, s * HW + lo:s * HW + hi])
            nc.scalar.dma_start(xar[:, 0:2048], xar_d[:, 0:2048])
            nc.sync.dma_start(xar[:, 2048:4096], xar_d[:, 2048:4096])

            lng = const.tile([C, 1], F32, tag="lng")
            lnb = const.tile([C, 1], F32, tag="lnb")
            nc.sync.dma_start(lng[:], lng_d[:])
            nc.sync.dma_start(lnb[:], lnb_d[:])
            fw1 = const.tile([2 * C, 9, C], BF16, tag="fw1")
            nc.sync.dma_start(fw1[:], fw1_d[:].rearrange("p (t o) -> p t o", t=9))
            msub = const.tile([C, C], F32R, tag="msub")
            lnm = const.tile([C, C], BF16, tag="lnm")
            nc.sync.dma_start(msub[:], msub_d[:])
            nc.sync.dma_start(lnm[:], lnm_d[:])
            bng = const.tile([C, 1], F32, tag="bng")
            bnb = const.tile([C, 1], F32, tag="bnb")
            fw2 = const.tile([C, C], F32R, tag="fw2")
            fb2 = const.tile([C, 1], F32, tag="fb2")
            nc.sync.dma_start(bng[:], bng_d[:])
            nc.sync.dma_start(bnb[:], bnb_d[:])
            nc.sync.dma_start(fw2[:], fw2_d[:])
            nc.sync.dma_start(fb2[:], fb2_d[:])

            eps = const.tile([C, 1], F32, tag="eps")
            nc.vector.memset(eps[:], EPS_LN)
            mka = const.tile([2 * C, 1], F32, tag="mka")
            mkb = const.tile([2 * C, 1], F32, tag="mkb")
            nc.sync.dma_start(mka[:], mka_d[:])
            nc.sync.dma_start(mkb[:], mkb_d[:])
            # zero padding rows of the conv windows: block0 w=0 (global -1)
            # and block1 w=33,34 (global 64,65)
            zr = const.tile([C, 128], BF16, tag="zr")
            nc.vector.memset(zr[:], 0.0)
            nc.sync.dma_start(a2a_in[0][:, 0:64], zr[:, 0:64])
            # band 1 block1: w=33,34 (global rows 64,65) at offsets 17,18
            nc.sync.dma_start(a2a_in[1][:, (19 + 17) * 64:(19 + 19) * 64],
                              zr[:])

            for rep in range(reps):
              actx = ExitStack()
              abig = actx.enter_context(tc.tile_pool(name=f"abig{rep}", bufs=1))
              ppool = actx.enter_context(tc.tile_pool(name=f"ppool{rep}", bufs=2))

              kpp = abig.tile([33, 2, HW], FP8, tag="kpp")  # 64*K'' pair-layout

              def emit_kproj(jt, n=1):   # K'' j-tiles jt..jt+n-1
                  for q in range(n):
                      sl = slice((jt + q) * 512, (jt + q + 1) * 512)
                      pk = lps.tile([48, 1024], F32, tag="lgt",
                                    name=f"pk{jt + q}")
                      for s in range(2):
                          nc.tensor.matmul(
                              pk[:, s * 512:(s + 1) * 512],
                              skp[:, :, s * 48:(s + 1) * 48],
                              xb8[:, :, sl], perf_mode=DR)
                      # slot-major fp8 store: [33,(2,strideHW),(512,1)] dest
                      nc.vector.tensor_scalar(
                          kpp[:, :, sl], pk[0:33, :], 1.0, 0.0, MULT, ADD)

              vtab = {}   # i-block -> (tile, col0)

              def emit_pv(b0, n):   # V for i-blocks b0..b0+n-1 (one grant)
                  pvp = lps.tile([128, 64 * n], F32, tag="lgt", name=f"pv{b0}")
                  for q in range(n):
                      ibb = b0 + q
                      nc.tensor.matmul(pvp[:, q * 64:(q + 1) * 64],
                                       xb8[:, :, ibb * 128:(ibb + 1) * 128],
                                       wva[:], perf_mode=DR)
                  vt = small.tile([128, 64 * n], F32, tag=f"vt{n}")
                  nc.vector.tensor_scalar(vt[:], pvp[:], 1.0, 0.0, MULT, ADD)
                  for q in range(n):
                      vtab[b0 + q] = (vt, q * 64)

              accb = [acc.tile([128, 512], F32, tag=f"acc{jj}", name=f"acc{jj}")
                      for jj in range(4)]

              PAs = {}
              vss = {}
              hold = {}

              def emit_qk(k, ch):
                  isl = slice(k * 128, (k + 1) * 128)
                  ps = lps.tile([128, 1024], F32, tag="lgt", name=f"ps{k}_{ch}")
                  c0 = ch * 1024
                  for hh in range(2):
                      sl = slice(c0 + hh * 512, c0 + (hh + 1) * 512)
                      nc.tensor.matmul(ps[:, hh * 512:(hh + 1) * 512],
                                       kpp[:, :, isl], xa8[:, :, sl],
                                       perf_mode=DR)
                  return ps

              def emit_exp(k, ch, ps):
                  if ch == 0:
                      PAs[k] = ppool.tile([128, HW], BF16, tag="PA",
                                          name=f"PA{k}")
                  nc.scalar.activation(PAs[k][:, ch * 1024:(ch + 1) * 1024],
                                       ps[:], Exp, scale=0.125 / 64)
                  if ch == 3:
                      S = small.tile([128, 1], F32, tag="S")
                      nc.vector.tensor_scalar(PAs[k][:], PAs[k][:], 1.0, 0.0,
                                              MULT, ADD, accum_out=S[:])
                      hold["S"] = S
                      R = small.tile([128, 1], F32, tag="R")
                      nc.vector.reciprocal(R[:], S[:])
                      vs = small.tile([128, C], BF16, tag="vs")
                      vt, c0v = vtab[k]
                      # vt carries 16*v; vs = (16 v) * R * 256 = 4096 v / S
                      nc.vector.tensor_scalar(
                          vs[:], vt[:, c0v:c0v + 64], R[:], 256.0, MULT, MULT)
                      vss[k] = vs

              def emit_av(k, ch, stop=False):
                  se = slice(ch * 1024, ch * 1024 + 512)
                  so = slice(ch * 1024 + 512, ch * 1024 + 1024)
                  first = (k == 0)
                  nc.tensor.matmul(accb[ch][0:C, :], vss[k][:], PAs[k][:, se],
                                   tile_position=(0, 0), start=first,
                                   stop=False)
                  nc.tensor.matmul(accb[ch][C:2 * C, :], vss[k][:],
                                   PAs[k][:, so],
                                   tile_position=(0, 64), start=first,
                                   stop=stop)

              emit_kproj(0)
              emit_kproj(1)
              emit_pv(0, 4)
              emit_pv(4, 4)

              pending = None
              for k in range(NI):
                  for ch in range(4):
                      ps = emit_qk(k, ch)
                      if pending is not None:
                          pk_, pch_ = pending[0], pending[1]
                          emit_exp(*pending)
                          if pk_ >= 1:
                              if pch_ == 0:
                                  emit_av(pk_ - 1, 0)
                              elif pch_ == 1:
                                  emit_av(pk_ - 1, 1)
                              elif pch_ == 2:
                                  emit_av(pk_ - 1, 2)
                                  emit_av(pk_ - 1, 3)
                      pending = (k, ch, ps)
                      if ch == 2 and k % 8 == 6 and k < 24:
                          t = k // 8
                          emit_kproj(2 * t + 2, n=2)
                          emit_pv(8 * t + 8, 8)
              emit_exp(*pending)
              # pre-swap the activation table to the sqrt set while the AV
              # drain runs, so LN's first Sqrt doesn't stall mid-pipeline.
              # Reading the last block's softmax sum pins this to the drain
              # (a dependency-free op would be scheduler-hoisted mid-loop).
              warm2 = small.tile([128, 1], F32, tag="warm2")
              nc.scalar.activation(warm2[:], hold["S"][:], Sqrt)
              for ch in range(4):
                  emit_av(NI - 1, ch, stop=True)

              actx.close()
              tctx = ExitStack()
              tmp = tctx.enter_context(tc.tile_pool(name=f"tmp{rep}", bufs=2))

              att = big.tile([C, HW], F32R, tag="att")

              # residual adds (psum + xa) must run on DVE (GPSIMD cannot
              # touch PSUM on real HW).  Emit 6,7 first so acc3 frees early
              # for the LN variance accumulator, then weave the rest between
              # LN chunks so the DVE FIFO stays fed.
              def emit_resid(jc):
                  jp, hh = jc // 2, jc % 2
                  s2 = slice(jc * 512, (jc + 1) * 512)
                  nc.vector.tensor_tensor(
                      att[:, s2], accb[jp][hh * C:(hh + 1) * C, :],
                      xar[:, s2], ADD)

              for jc in [6, 7, 0, 1]:
                  emit_resid(jc)

              cinb = [big.tile([2 * C, WLEN[p], W], BF16, tag=f"cin{p}",
                               name=f"cinb{p}")
                      for p in range(NBAND)]
              y = big.tile([C, HW // 2], BF16, tag="y")
              bnp4 = small.tile([C, 4], F32, tag="bnp4")
              bnq4 = small.tile([C, 4], F32, tag="bnq4")

              def emit_ln_half(jc):
                  sl = slice(jc * 512, (jc + 1) * 512)
                  pxm = acc.tile([C, 512], F32, tag=f"acc{jc % 3}",
                                 name=f"pxm{jc}")
                  nc.tensor.matmul(pxm[:], msub[:], att[:, sl])
                  sq2 = tmp.tile([C, 512], BF16, tag="sq2")
                  nc.scalar.activation(sq2[:], pxm[:], Square)
                  if jc % 3 == 0:
                      pe2 = acc.tile([C, 512], F32, tag="acc3",
                                     name=f"pe2{jc}")
                  else:
                      pe2 = lps.tile([C, 512], F32, tag="lgt",
                                     name=f"pe2{jc}")
                  nc.tensor.matmul(pe2[:], lnm[:], sq2[:])
                  sd = tmp.tile([C, 512], F32, tag="sd")
                  nc.scalar.activation(sd[:], pe2[:], Sqrt, bias=eps[:])
                  rstd = tmp.tile([C, 512], F32, tag="rstd")
                  nc.vector.reciprocal(rstd[:], sd[:])
                  xh = tmp.tile([C, 512], BF16, tag="xh")
                  nc.vector.tensor_tensor(xh[:], pxm[:], rstd[:], MULT)
                  oln = tmp.tile([C, 512], BF16, tag="oln")
                  nc.vector.tensor_scalar(oln[:], xh[:], lng[:], lnb[:],
                                          MULT, ADD)
                  # scatter this chunk of LN output into the AllToAll send
                  # buffer: block h = the 35-row conv window of rank h
                  # (window row w <-> global row 32h - 1 + w)
                  for h in range(2):
                      g0b = max(8 * jc, 0 if h == 0 else 31)
                      g1b = min(8 * jc + 8, 34 if h == 0 else 64)
                      for p in range(NBAND):
                          w0p, w1p = WSTART[p], WSTART[p] + WLEN[p]
                          gof = -1 if h == 0 else 31    # g = w + gof
                          g0 = max(g0b, w0p + gof)
                          g1 = min(g1b, w1p + gof)
                          if g0 >= g1:
                              continue
                          w0 = g0 - gof
                          base = h * WLEN[p] * 64 + (w0 - w0p) * 64
                          dst = a2a_in[p][:, base:base + (g1 - g0) * 64]
                          src = oln[:, (g0 - 8 * jc) * 64:
                                    (g1 - 8 * jc) * 64]
                          nc.sync.dma_start(dst, src)

              def emit_a2a(p):
                  if fake_cc:
                      nc.scalar.dma_start(a2a_out[p][0:C, :], a2a_in[p][:])
                      nc.sync.dma_start(a2a_out[p][C:2 * C, :], a2a_in[p][:])
                  else:
                      nc.gpsimd.collective_compute(
                          "AllGather", mybir.AluOpType.bypass,
                          replica_groups=AG_GROUPS,
                          ins=[a2a_in[p][:]], outs=[a2a_out[p][:]])

              wabs = {}

              def emit_wab_load(p):
                  wl64 = WLEN[p] * 64
                  wab = tmp.tile([2 * C, 2 * 19 * 64], BF16, tag="wab",
                                 name=f"wab{p}")
                  nc.scalar.dma_start(wab[:, 0:2 * wl64], a2a_out[p][:])
                  wabs[p] = wab

              def emit_cin_sel(p):
                  wl64 = WLEN[p] * 64
                  wab = wabs[p]
                  ta = tmp.tile([2 * C, 19 * 64], BF16, tag="ta")
                  nc.vector.tensor_scalar_mul(ta[:, 0:wl64],
                                              wab[:, 0:wl64], mka[:])
                  tb = tmp.tile([2 * C, 19 * 64], BF16, tag="tb")
                  nc.vector.tensor_scalar_mul(tb[:, 0:wl64],
                                              wab[:, wl64:2 * wl64], mkb[:])
                  cbf = cinb[p][:].rearrange("p h w -> p (h w)")
                  nc.vector.tensor_tensor(cbf[:, 0:wl64],
                                          ta[:, 0:wl64], tb[:, 0:wl64], ADD)

              TAPS = [(1, 1)] + [(ki, kj) for ki in range(3) for kj in range(3)
                                 if (ki, kj) != (1, 1)]

              def emit_conv_group(g):
                  pc = acc.tile([C, 8, W], F32, tag=f"acc{g}", name=f"pc{g}")
                  for t, (ki, kj) in enumerate(TAPS):
                      w_lo = max(0, 1 - kj)
                      w_hi = min(W, W + 1 - kj)
                      rhs = cinb[g // 2][:, 8 * (g % 2) + ki:
                                         8 * (g % 2) + ki + 8,
                                         w_lo + kj - 1:w_hi + kj - 1]
                      nc.tensor.matmul(pc[:, 0:8, w_lo:w_hi],
                                       fw1[:, 3 * ki + kj, :], rhs,
                                       start=(t == 0), stop=(t == 8))
                  return pc

              for jc in range(8):
                  emit_ln_half(jc)
                  if jc < 4:
                      emit_resid(jc + 2)
              for p in range(NBAND):
                  emit_a2a(p)
              for p in range(NBAND):
                  emit_wab_load(p)
              # PE p-state warm-up: dummy matmuls gated on the exchange
              # load, so the tensor engine is ramping exactly when the conv
              # input lands (otherwise conv starts at the slow pstate)
              for d in range(4):
                  pw = lps.tile([C, 512], F32, tag="lgt", name=f"pw{d}")
                  nc.tensor.matmul(pw[:], lnm[:],
                                   wabs[0][0:C, d * 512:(d + 1) * 512])
              for p in range(NBAND):
                  emit_cin_sel(p)
              for g in range(4):
                  pc = emit_conv_group(g)
                  ysl = y[:, g * 512:(g + 1) * 512]
                  nc.vector.tensor_scalar(
                      ysl, pc[:].rearrange("p r w -> p (r w)"), 1.0, 0.0,
                      MULT, ADD, accum_out=bnp4[:, g:g + 1])
                  ysq = tmp.tile([C, 512], BF16, tag="ysq")
                  nc.scalar.activation(ysq[:], ysl, Square,
                                       accum_out=bnq4[:, g:g + 1])

              # ---- BatchNorm stats (cross-sample AllReduce, all 8 cores
              # hold disjoint row-halves) ----
              bnp = small.tile([C, 2], F32, tag="bnp")
              nc.vector.tensor_reduce(bnp[:, 0:1], bnp4[:], AX, ADD)
              nc.vector.tensor_reduce(bnp[:, 1:2], bnq4[:], AX, ADD)
              nc.sync.dma_start(ar_in[:], bnp[:])
              if fake_cc:
                  nc.sync.dma_start(ar_out[:], ar_in[:])
              else:
                  nc.gpsimd.collective_compute(
                      "AllReduce", mybir.AluOpType.add,
                      replica_groups=AR_GROUPS,
                      ins=[ar_in[:]], outs=[ar_out[:]])
              bns = small.tile([C, 2], F32, tag="bns")
              nc.sync.dma_start(bns[:], ar_out[:])

              m2 = small.tile([C, 2], F32, tag="m2")
              nc.vector.tensor_scalar_mul(m2[:], bns[:], 1.0 / BN_COUNT)
              musq2 = small.tile([C, 1], F32, tag="musq2")
              nc.vector.tensor_mul(musq2[:], m2[:, 0:1], m2[:, 0:1])
              varb = small.tile([C, 1], F32, tag="varb")
              nc.vector.tensor_sub(varb[:], m2[:, 1:2], musq2[:])
              sdb = small.tile([C, 1], F32, tag="sdb")
              nc.scalar.activation(sdb[:], varb[:], Sqrt, bias=eps[:])
              rstdb = small.tile([C, 1], F32, tag="rstdb")
              nc.vector.reciprocal(rstdb[:], sdb[:])
              scl = small.tile([C, 1], F32, tag="scl")
              nc.vector.tensor_mul(scl[:], bng[:], rstdb[:])
              msc = small.tile([C, 1], F32, tag="msc")
              nc.vector.tensor_mul(msc[:], m2[:, 0:1], scl[:])
              shf = small.tile([C, 1], F32, tag="shf")
              nc.vector.tensor_sub(shf[:], bnb[:], msc[:])

              # ---- BN apply + ReLU + final 1x1 conv (own row-half) ----
              yr = big.tile([C, HW // 2], F32R, tag="yr")
              for hh in range(2):
                  sl = slice(hh * 1024, (hh + 1) * 1024)
                  nc.scalar.activation(yr[:, sl], y[:, sl], Relu,
                                       scale=scl[:], bias=shf[:])
                  po = lps.tile([C, 1024], F32, tag="lgt", name=f"po{hh}")
                  for q in range(2):
                      s2 = slice(hh * 1024 + q * 512,
                                 hh * 1024 + (q + 1) * 512)
                      nc.tensor.matmul(po[:, q * 512:(q + 1) * 512],
                                       fw2[:], yr[:, s2])
                  ot = tmp.tile([C, 1024], F32, tag="ot")
                  nc.vector.tensor_scalar_add(ot[:], po[:], fb2[:])
                  [nc.sync, nc.scalar][hh].dma_start(out_d[:, sl], ot[:])
              tctx.close()

    nc.compile()
    return nc


def _get_nc(reps=1, fake_cc=False):
    key = f"nc{reps}_{fake_cc}"
    if key not in _CACHE:
        _CACHE[key] = _build(reps=reps, fake_cc=fake_cc)
    return _CACHE[key]


def _make_in_maps(inputs):
    return _build_in_maps(**inputs)


def _pack_pairs(m):
    """[66, N] fp32 -> [33, 2N] fp8 slot-major: channel s*33+p at [p, s*N+j]."""
    n = m.shape[1]
    return np.ascontiguousarray(
        m.reshape(2, 33, n).transpose(1, 0, 2).reshape(33, 2 * n)
    ).astype(ml_dtypes.float8_e4m3)


def _to66(m):
    out = np.zeros((66, m.shape[1]), np.float32)
    out[:m.shape[0]] = m
    return out


def _build_in_maps(x_s2, x_dem, wq1, bq1, wk1, bk1, wv1, bv1,
                   wq2, bq2, wk2, bk2, wv2, bv2,
                   ln_s2_w, ln_s2_b, ln_dem_w, ln_dem_b,
                   fw1, fb1, bn_g, bn_b, fw2, fb2):
    f32 = np.float32
    x_s2 = np.asarray(x_s2, f32).reshape(B, C, HW)
    x_dem = np.asarray(x_dem, f32).reshape(B, C, HW)

    fw1t = np.ascontiguousarray(
        np.transpose(np.asarray(fw1, f32), (1, 2, 3, 0)).reshape(2 * C, 9 * C)
    ).astype(ml_dtypes.bfloat16)
    msub = (np.eye(C, dtype=f32) - np.full((C, C), 1.0 / C, f32))
    lnm16 = np.full((C, C), 1.0 / C, f32).astype(ml_dtypes.bfloat16)
    common = {
        "msub": msub,
        "lnm16": lnm16,
        "fw1t": fw1t,
        "bng": np.asarray(bn_g, f32).reshape(C, 1),
        "bnb": np.asarray(bn_b, f32).reshape(C, 1),
        "fw2T": np.ascontiguousarray(np.asarray(fw2, f32).T),
        "fb2": np.asarray(fb2, f32).reshape(C, 1),
    }

    def mk_dir(wq, bq, wk, bk, wv, bv, lg, lb):
        wq = np.asarray(wq, f32); wk = np.asarray(wk, f32)
        wv = np.asarray(wv, f32)
        bq = np.asarray(bq, f32).reshape(C)
        bk = np.asarray(bk, f32).reshape(C)
        bv = np.asarray(bv, f32).reshape(C)
        skp = np.zeros((66, 66), f32)
        skp[:C, :C] = wk.T @ wq
        skp[C, :C] = wq.T @ bk
        skp[:C, C] = wk.T @ bq
        skp[C, C] = bk @ bq
        wva = np.zeros((66, C), f32)
        wva[:C, :] = wv.T
        wva[C, :] = bv
        # x64 / x16 pre-scales keep the fp8 mantissas in the normal range;
        # undone by exp scale 0.125/64 and the vs *256 normalization.
        skp_p = _pack_pairs(skp * 64.0).reshape(33, 2, 66)
        skp96 = np.zeros((33, 2, 96), skp_p.dtype)
        skp96[:, :, 0:33] = skp_p[:, :, 0:33]
        skp96[:, :, 48:81] = skp_p[:, :, 33:66]
        return dict(skp8=skp96.reshape(33, 192),
                    wva8=_pack_pairs(wva * 16.0),
                    lng=np.asarray(lg, f32).reshape(C, 1),
                    lnb=np.asarray(lb, f32).reshape(C, 1))

    dir_params = [
        mk_dir(wq1, bq1, wk1, bk1, wv1, bv1, ln_s2_w, ln_s2_b),
        mk_dir(wq2, bq2, wk2, bk2, wv2, bv2, ln_dem_w, ln_dem_b),
    ]
    x8 = {}
    for b in range(B):
        for nm, xv in (("s2", x_s2[b]), ("dem", x_dem[b])):
            x66 = _to66(xv)
            x66[C] = 1.0
            x8[(b, nm)] = _pack_pairs(x66)
    in_maps = []
    for c in range(N_CORES):
        b, d = c // 2, c % 2
        xaq = x_s2[b] if d == 0 else x_dem[b]
        m = {"xa8": x8[(b, "s2" if d == 0 else "dem")],
             "xb8": x8[(b, "dem" if d == 0 else "s2")],
             "xar": np.ascontiguousarray(xaq) * 4096.0,
             "mka": np.full((2 * C, 1), 1.0 - d, f32),
             "mkb": np.full((2 * C, 1), float(d), f32)}
        m.update(dir_params[d])
        m.update(common)
        in_maps.append(m)
    return in_maps


def kernel(**inputs):
    nc = _get_nc()
    in_maps = _make_in_maps(inputs)
    res = run_bass_kernel_spmd(nc, in_maps, list(range(N_CORES)))
    out = np.empty((B, C, H, W), np.float32)
    for b in range(B):
        half = np.concatenate([res.results[2 * b]["out"],
                               res.results[2 * b + 1]["out"]], axis=1)
        out[b] = half.reshape(C, H, W)
    return out

